# revision 1
# baseline (speedup 1.0000x reference)
"""Trainium2 Bass kernel: per-(b,c) exponential moving average along T.

Reference semantics (fp32):
    w   = clip(weights, 0.02, 1.0)            # [C]
    y[:, :, 0] = w*x0 + (1-w)*x0              # init acc = x[:, :, 0]
    y[:, :, t] = w*x[:, :, t] + (1-w)*y[:, :, t-1]

Kernel formulation (per core, C=128 channels on partitions, T on free axis):
    z_t = a*z_{t-1} + x_t   with z_{-1} = x_0 / w   (DVE tensor_tensor_scan)
    y_t = w * z_t                                   (ACT per-partition scale)

Sharding: batch dim B=32 split across 8 cores (4 batches each); weights are
replicated. No cross-core communication.
"""

import numpy as np
from contextlib import ExitStack

import concourse.bacc as bacc
import concourse.tile as tile
from concourse import mybir
from concourse.bass_utils import run_bass_kernel_spmd

B, C, T = 32, 128, 16384
N_CORES = 8
BPC = B // N_CORES  # batches per core
FT = 4096           # free-dim tile (per DMA / per scan instruction)

F32 = mybir.dt.float32


def build_nc(bpc=BPC, c=C, t=T, ft=FT, debug=False):
    nt = t // ft
    assert t % ft == 0
    nc = bacc.Bacc(
        "TRN2", target_bir_lowering=False, debug=debug, num_devices=N_CORES
    )
    x_in = nc.dram_tensor("x", [bpc, c, t], F32, kind="ExternalInput")
    w_in = nc.dram_tensor("w", [c, 1], F32, kind="ExternalInput")
    a_in = nc.dram_tensor("a", [c, 1], F32, kind="ExternalInput")
    wi_in = nc.dram_tensor("wi", [c, 1], F32, kind="ExternalInput")
    y_out = nc.dram_tensor("y", [bpc, c, t], F32, kind="ExternalOutput")

    with tile.TileContext(nc) as tc:
        with ExitStack() as ctx:
            const = ctx.enter_context(tc.tile_pool(name="const", bufs=1))
            xp = ctx.enter_context(tc.tile_pool(name="xp", bufs=4))
            zp = ctx.enter_context(tc.tile_pool(name="zp", bufs=4))
            cp = ctx.enter_context(tc.tile_pool(name="cp", bufs=4))

            w_t = const.tile([c, 1], F32, tag="w")
            a_t = const.tile([c, 1], F32, tag="a")
            wi_t = const.tile([c, 1], F32, tag="wi")
            nc.sync.dma_start(w_t[:], w_in[:])
            nc.sync.dma_start(a_t[:], a_in[:])
            nc.sync.dma_start(wi_t[:], wi_in[:])

            # a broadcast along the free axis for the scan's data0 operand
            ones = const.tile([c, ft], F32, tag="ones")
            nc.vector.memset(ones[:], 1.0)
            a_full = const.tile([c, ft], F32, tag="a_full")
            nc.scalar.mul(a_full[:], ones[:], a_t[:])

            for b in range(bpc):
                init = None
                for k in range(nt):
                    xt = xp.tile([c, ft], F32, tag="xt")
                    nc.sync.dma_start(xt[:], x_in[b, :, k * ft:(k + 1) * ft])
                    if k == 0:
                        # z_{-1} = x0 / w  so that y0 = w*(a*z_{-1} + x0) = x0
                        init = cp.tile([c, 1], F32, tag="init")
                        nc.vector.tensor_scalar_mul(init[:], xt[:, 0:1], wi_t[:])
                    zt = zp.tile([c, ft], F32, tag="zt")
                    nc.vector.tensor_tensor_scan(
                        out=zt[:],
                        data0=a_full[:],
                        data1=xt[:],
                        initial=init[:],
                        op0=mybir.AluOpType.mult,
                        op1=mybir.AluOpType.add,
                    )
                    if k < nt - 1:
                        # carry z's last column out before the in-place scale
                        init = cp.tile([c, 1], F32, tag="init")
                        nc.vector.tensor_copy(init[:], zt[:, ft - 1:ft])
                    nc.scalar.mul(zt[:], zt[:], w_t[:])  # y = w*z (in place)
                    nc.sync.dma_start(y_out[b, :, k * ft:(k + 1) * ft], zt[:])
    nc.compile()
    return nc


_NC_CACHE = None


def _get_nc():
    global _NC_CACHE
    if _NC_CACHE is None:
        _NC_CACHE = build_nc()
    return _NC_CACHE


def make_in_maps(x, weights):
    x = np.asarray(x, dtype=np.float32)
    w = np.clip(np.asarray(weights, dtype=np.float32), 0.02, 1.0).astype(np.float32)
    a = (np.float32(1.0) - w).astype(np.float32)
    wi = (np.float32(1.0) / w).astype(np.float32)
    in_maps = []
    for i in range(N_CORES):
        in_maps.append(
            {
                "x": np.ascontiguousarray(x[i * BPC:(i + 1) * BPC]),
                "w": w.reshape(C, 1),
                "a": a.reshape(C, 1),
                "wi": wi.reshape(C, 1),
            }
        )
    return in_maps


def kernel(x, weights):
    nc = _get_nc()
    in_maps = make_in_maps(x, weights)
    res = run_bass_kernel_spmd(nc, in_maps, list(range(N_CORES)))
    return np.concatenate([r["y"] for r in res.results], axis=0)


# revision 17
# speedup vs baseline: 79436.3054x; 79436.3054x over previous
"""Trainium2 Bass kernel: per-(b,c) exponential moving average along T.

Reference semantics (fp32):
    w   = clip(weights, 0.02, 1.0)            # [C]
    y[:, :, 0] = w*x0 + (1-w)*x0              # init acc = x[:, :, 0]
    y[:, :, t] = w*x[:, :, t] + (1-w)*y[:, :, t-1]

Kernel formulation (per core, C=128 channels on partitions, T on free axis):
    z_t = a*z_{t-1} + x_t   with z_{-1} = x_0 / w   (DVE tensor_tensor_scan)
    y_t = w * z_t                                   (ACT per-partition scale)

Sharding: batch dim B=32 split across 8 cores (4 batches each); weights are
replicated. No cross-core communication.
"""

import numpy as np
from contextlib import ExitStack

import concourse.bacc as bacc
import concourse.tile as tile
from concourse import mybir
from concourse.bass_utils import run_bass_kernel_spmd

B, C, T = 32, 128, 16384
N_CORES = 8
BPC = B // N_CORES  # batches per core
FT = 8192           # free-dim tile (per DMA / per scan instruction)

F32 = mybir.dt.float32


def build_nc(
    bpc=BPC,
    c=C,
    t=T,
    ft=FT,
    debug=False,
    loop_k=1,
    bufs_x=3,
    bufs_z=2,
    bcast_a=False,
    store_eng="sync",
    sizes=None,
):
    if sizes is None:
        assert t % ft == 0
        sizes = [ft] * (t // ft)
    sizes = list(sizes)
    assert sum(sizes) == t
    ft = max(sizes)
    nt = len(sizes)
    nc = bacc.Bacc(
        "TRN2", target_bir_lowering=False, debug=debug, num_devices=N_CORES
    )
    x_in = nc.dram_tensor("x", [bpc, c, t], F32, kind="ExternalInput")
    w_in = nc.dram_tensor("w", [c, 1], F32, kind="ExternalInput")
    a_in = nc.dram_tensor("a", [c, 1], F32, kind="ExternalInput")
    wi_in = nc.dram_tensor("wi", [c, 1], F32, kind="ExternalInput")
    y_out = nc.dram_tensor("y", [bpc, c, t], F32, kind="ExternalOutput")

    store = {"sync": nc.sync, "scalar": nc.scalar, "gpsimd": nc.gpsimd}[store_eng]

    with tile.TileContext(nc) as tc:
        with ExitStack() as ctx:
            const = ctx.enter_context(tc.tile_pool(name="const", bufs=1))
            xp = ctx.enter_context(tc.tile_pool(name="xp", bufs=bufs_x))
            zp = ctx.enter_context(tc.tile_pool(name="zp", bufs=bufs_z))
            cp = ctx.enter_context(tc.tile_pool(name="cp", bufs=4))

            w_t = const.tile([c, 1], F32, tag="w")
            a_t = const.tile([c, 1], F32, tag="a")
            wi_t = const.tile([c, 1], F32, tag="wi")
            nc.sync.dma_start(w_t[:], w_in[:])
            nc.sync.dma_start(a_t[:], a_in[:])
            nc.sync.dma_start(wi_t[:], wi_in[:])

            # a broadcast along the free axis for the scan's data0 operand
            if bcast_a:
                a_full_ap = a_t[:].broadcast_to([c, ft])
            else:
                a_full = const.tile([c, ft], F32, tag="a_full")
                nc.vector.memset(a_full[:], 1.0)
                nc.scalar.mul(a_full[:], a_full[:], a_t[:])
                a_full_ap = a_full[:]

            def body():
                for b in range(bpc):
                    init = None
                    off = 0
                    for k, fk in enumerate(sizes):
                        xt = xp.tile([c, ft], F32, tag="xt")
                        nc.sync.dma_start(
                            xt[:, :fk], x_in[b, :, off:off + fk]
                        )
                        if k == 0:
                            # z_{-1} = x0 / w  so that y0 = w*(a*z_{-1}+x0) = x0
                            init = cp.tile([c, 1], F32, tag="init")
                            nc.vector.tensor_scalar_mul(
                                init[:], xt[:, 0:1], wi_t[:]
                            )
                        zt = zp.tile([c, ft], F32, tag="zt")
                        nc.vector.tensor_tensor_scan(
                            out=zt[:, :fk],
                            data0=a_full_ap[:, :fk],
                            data1=xt[:, :fk],
                            initial=init[:],
                            op0=mybir.AluOpType.mult,
                            op1=mybir.AluOpType.add,
                        )
                        if k < nt - 1:
                            # carry z's last column before the in-place scale
                            init = cp.tile([c, 1], F32, tag="init")
                            nc.vector.tensor_copy(init[:], zt[:, fk - 1:fk])
                        nc.scalar.mul(zt[:, :fk], zt[:, :fk], w_t[:])  # y = w*z
                        store.dma_start(
                            y_out[b, :, off:off + fk], zt[:, :fk]
                        )
                        off += fk

            if loop_k > 1:
                # timing-only variant: repeat the whole pass on-device
                with tc.For_i(0, loop_k, 1):
                    body()
            else:
                body()
    nc.compile()
    return nc


def build_nc_merged(
    bpc=BPC,
    c=C,
    t=T,
    g=2,
    ft=8192,
    bufs=2,
    debug=False,
    loop_k=1,
    store_eng="sync",
):
    """Merged variant: one SBUF tile holds `g` batches x `ft` columns, loaded
    and stored as a single large DMA; the scan and the w-scale run in place
    over the tile (no separate z pool)."""
    nt = t // ft
    ng = bpc // g
    assert t % ft == 0 and bpc % g == 0
    nc = bacc.Bacc(
        "TRN2", target_bir_lowering=False, debug=debug, num_devices=N_CORES
    )
    x_in = nc.dram_tensor("x", [bpc, c, t], F32, kind="ExternalInput")
    w_in = nc.dram_tensor("w", [c, 1], F32, kind="ExternalInput")
    a_in = nc.dram_tensor("a", [c, 1], F32, kind="ExternalInput")
    wi_in = nc.dram_tensor("wi", [c, 1], F32, kind="ExternalInput")
    y_out = nc.dram_tensor("y", [bpc, c, t], F32, kind="ExternalOutput")

    store = {"sync": nc.sync, "scalar": nc.scalar, "gpsimd": nc.gpsimd}[store_eng]

    with tile.TileContext(nc) as tc:
        with ExitStack() as ctx:
            const = ctx.enter_context(tc.tile_pool(name="const", bufs=1))
            xp = ctx.enter_context(tc.tile_pool(name="xp", bufs=bufs))
            cp = ctx.enter_context(tc.tile_pool(name="cp", bufs=2 * bpc))

            w_t = const.tile([c, 1], F32, tag="w")
            a_t = const.tile([c, 1], F32, tag="a")
            wi_t = const.tile([c, 1], F32, tag="wi")
            nc.sync.dma_start(w_t[:], w_in[:])
            nc.sync.dma_start(a_t[:], a_in[:])
            nc.sync.dma_start(wi_t[:], wi_in[:])

            a_full = const.tile([c, ft], F32, tag="a_full")
            nc.vector.memset(a_full[:], 1.0)
            nc.scalar.mul(a_full[:], a_full[:], a_t[:])

            def body():
                for gi in range(ng):
                    carry = [None] * g
                    for k in range(nt):
                        xt = xp.tile([c, g * ft], F32, tag="xt")
                        src = x_in[gi * g:(gi + 1) * g, :, k * ft:(k + 1) * ft]
                        dst = xt[:].rearrange("c (g f) -> c g f", g=g)
                        nc.sync.dma_start(dst, src.transpose([1, 0, 2]))
                        for j in range(g):
                            seg = xt[:, j * ft:(j + 1) * ft]
                            if k == 0:
                                init = cp.tile([c, 1], F32, tag="init")
                                nc.vector.tensor_scalar_mul(
                                    init[:], xt[:, j * ft:j * ft + 1], wi_t[:]
                                )
                                carry[j] = init
                            nc.vector.tensor_tensor_scan(
                                out=seg,
                                data0=a_full[:],
                                data1=seg,
                                initial=carry[j][:],
                                op0=mybir.AluOpType.mult,
                                op1=mybir.AluOpType.add,
                            )
                            if k < nt - 1:
                                init = cp.tile([c, 1], F32, tag="init")
                                nc.vector.tensor_copy(
                                    init[:], xt[:, (j + 1) * ft - 1:(j + 1) * ft]
                                )
                                carry[j] = init
                        nc.scalar.mul(xt[:], xt[:], w_t[:])  # y = w*z in place
                        out_dst = y_out[gi * g:(gi + 1) * g, :, k * ft:(k + 1) * ft]
                        store.dma_start(
                            out_dst.transpose([1, 0, 2]),
                            xt[:].rearrange("c (g f) -> c g f", g=g),
                        )

            if loop_k > 1:
                with tc.For_i(0, loop_k, 1):
                    body()
            else:
                body()
    nc.compile()
    return nc


def build_nc_prescale(
    bpc=BPC,
    c=C,
    t=T,
    g=1,
    ft=8192,
    bufs=4,
    debug=False,
    loop_k=1,
    store_eng="sync",
    seg_store=True,
):
    """In-place pre-scale variant: ACT computes wx in place over the loaded
    tile, DVE scans y = a*y + wx in place, and the store reads the scan
    output directly (per segment when seg_store)."""
    nt = t // ft
    ng = bpc // g
    assert t % ft == 0 and bpc % g == 0
    nc = bacc.Bacc(
        "TRN2", target_bir_lowering=False, debug=debug, num_devices=N_CORES
    )
    x_in = nc.dram_tensor("x", [bpc, c, t], F32, kind="ExternalInput")
    w_in = nc.dram_tensor("w", [c, 1], F32, kind="ExternalInput")
    a_in = nc.dram_tensor("a", [c, 1], F32, kind="ExternalInput")
    wi_in = nc.dram_tensor("wi", [c, 1], F32, kind="ExternalInput")
    y_out = nc.dram_tensor("y", [bpc, c, t], F32, kind="ExternalOutput")

    store = {"sync": nc.sync, "scalar": nc.scalar, "gpsimd": nc.gpsimd}[store_eng]

    with tile.TileContext(nc) as tc:
        with ExitStack() as ctx:
            const = ctx.enter_context(tc.tile_pool(name="const", bufs=1))
            xp = ctx.enter_context(tc.tile_pool(name="xp", bufs=bufs))
            cp = ctx.enter_context(tc.tile_pool(name="cp", bufs=2 * bpc))

            w_t = const.tile([c, 1], F32, tag="w")
            a_t = const.tile([c, 1], F32, tag="a")
            nc.sync.dma_start(w_t[:], w_in[:])
            nc.sync.dma_start(a_t[:], a_in[:])
            # wi is unused here but kept as an input so in_maps stay uniform
            wi_t = const.tile([c, 1], F32, tag="wi")
            nc.sync.dma_start(wi_t[:], wi_in[:])

            a_full = const.tile([c, ft], F32, tag="a_full")
            nc.vector.memset(a_full[:], 1.0)
            nc.scalar.mul(a_full[:], a_full[:], a_t[:])

            def body():
                for gi in range(ng):
                    carry = [None] * g
                    for k in range(nt):
                        xt = xp.tile([c, g * ft], F32, tag="xt")
                        if g == 1:
                            nc.sync.dma_start(
                                xt[:], x_in[gi, :, k * ft:(k + 1) * ft]
                            )
                        else:
                            src = x_in[
                                gi * g:(gi + 1) * g, :, k * ft:(k + 1) * ft
                            ]
                            nc.sync.dma_start(
                                xt[:].rearrange("c (g f) -> c g f", g=g),
                                src.transpose([1, 0, 2]),
                            )
                        if k == 0:
                            # y_{-1} = x0 so that y0 = a*x0 + w*x0 = x0
                            for j in range(g):
                                init = cp.tile([c, 1], F32, tag="init")
                                nc.vector.tensor_copy(
                                    init[:], xt[:, j * ft:j * ft + 1]
                                )
                                carry[j] = init
                        nc.scalar.mul(xt[:], xt[:], w_t[:])  # wx in place
                        for j in range(g):
                            seg = xt[:, j * ft:(j + 1) * ft]
                            nc.vector.tensor_tensor_scan(
                                out=seg,
                                data0=a_full[:],
                                data1=seg,
                                initial=carry[j][:],
                                op0=mybir.AluOpType.mult,
                                op1=mybir.AluOpType.add,
                            )
                            if k < nt - 1:
                                init = cp.tile([c, 1], F32, tag="init")
                                nc.vector.tensor_copy(
                                    init[:], xt[:, (j + 1) * ft - 1:(j + 1) * ft]
                                )
                                carry[j] = init
                            if seg_store:
                                store.dma_start(
                                    y_out[gi * g + j, :, k * ft:(k + 1) * ft],
                                    seg,
                                )
                        if not seg_store:
                            out_dst = y_out[
                                gi * g:(gi + 1) * g, :, k * ft:(k + 1) * ft
                            ]
                            store.dma_start(
                                out_dst.transpose([1, 0, 2]),
                                xt[:].rearrange("c (g f) -> c g f", g=g),
                            )

            if loop_k > 1:
                with tc.For_i(0, loop_k, 1):
                    body()
            else:
                body()
    nc.compile()
    return nc


def build_nc_sched(
    bpc=BPC,
    c=C,
    t=T,
    g=2,
    sizes=(2048, 4096, 8192, 2048),
    bufs=2,
    debug=False,
    loop_k=1,
):
    """Pre-scale in-place variant with a non-uniform k-step schedule: small
    first step (compute/stores start early) and small last step (short tail),
    large steps in the middle for DMA efficiency. All steps share one
    max-sized pool slot."""
    ng = bpc // g
    sizes = list(sizes)
    assert sum(sizes) == t and bpc % g == 0
    ftmax = max(sizes)
    nc = bacc.Bacc(
        "TRN2", target_bir_lowering=False, debug=debug, num_devices=N_CORES
    )
    x_in = nc.dram_tensor("x", [bpc, c, t], F32, kind="ExternalInput")
    w_in = nc.dram_tensor("w", [c, 1], F32, kind="ExternalInput")
    a_in = nc.dram_tensor("a", [c, 1], F32, kind="ExternalInput")
    wi_in = nc.dram_tensor("wi", [c, 1], F32, kind="ExternalInput")
    y_out = nc.dram_tensor("y", [bpc, c, t], F32, kind="ExternalOutput")

    with tile.TileContext(nc) as tc:
        with ExitStack() as ctx:
            const = ctx.enter_context(tc.tile_pool(name="const", bufs=1))
            xp = ctx.enter_context(tc.tile_pool(name="xp", bufs=bufs))
            cp = ctx.enter_context(tc.tile_pool(name="cp", bufs=2 * bpc))

            w_t = const.tile([c, 1], F32, tag="w")
            a_t = const.tile([c, 1], F32, tag="a")
            wi_t = const.tile([c, 1], F32, tag="wi")
            # consts via SWDGE so the sync HWDGE ring starts with x loads
            nc.gpsimd.dma_start(w_t[:], w_in[:])
            nc.gpsimd.dma_start(a_t[:], a_in[:])
            nc.gpsimd.dma_start(wi_t[:], wi_in[:])

            a_full = const.tile([c, ftmax], F32, tag="a_full")
            nc.vector.memset(a_full[:], 1.0)
            nc.scalar.mul(a_full[:], a_full[:], a_t[:])

            def body():
                for gi in range(ng):
                    carry = [None] * g
                    off = 0
                    for ki, fk in enumerate(sizes):
                        xt = xp.tile([c, g * ftmax], F32, tag="xt")
                        src = x_in[gi * g:(gi + 1) * g, :, off:off + fk]
                        nc.sync.dma_start(
                            xt[:, : g * fk].rearrange("c (g f) -> c g f", g=g),
                            src.transpose([1, 0, 2]),
                        )
                        if ki == 0:
                            for j in range(g):
                                init = cp.tile([c, 1], F32, tag="init")
                                nc.vector.tensor_copy(
                                    init[:], xt[:, j * fk:j * fk + 1]
                                )
                                carry[j] = init
                        nc.scalar.mul(xt[:, : g * fk], xt[:, : g * fk], w_t[:])
                        for j in range(g):
                            seg = xt[:, j * fk:(j + 1) * fk]
                            nc.vector.tensor_tensor_scan(
                                out=seg,
                                data0=a_full[:, :fk],
                                data1=seg,
                                initial=carry[j][:],
                                op0=mybir.AluOpType.mult,
                                op1=mybir.AluOpType.add,
                            )
                            if ki < len(sizes) - 1:
                                init = cp.tile([c, 1], F32, tag="init")
                                nc.vector.tensor_copy(
                                    init[:], xt[:, (j + 1) * fk - 1:(j + 1) * fk]
                                )
                                carry[j] = init
                            nc.sync.dma_start(
                                y_out[gi * g + j, :, off:off + fk], seg
                            )
                        off += fk

            if loop_k > 1:
                with tc.For_i(0, loop_k, 1):
                    body()
            else:
                body()
    nc.compile()
    return nc


_NC_CACHE = None


def _get_nc():
    global _NC_CACHE
    if _NC_CACHE is None:
        _NC_CACHE = build_nc()
    return _NC_CACHE


def make_in_maps(x, weights):
    x = np.asarray(x, dtype=np.float32)
    w = np.clip(np.asarray(weights, dtype=np.float32), 0.02, 1.0).astype(np.float32)
    a = (np.float32(1.0) - w).astype(np.float32)
    wi = (np.float32(1.0) / w).astype(np.float32)
    in_maps = []
    for i in range(N_CORES):
        in_maps.append(
            {
                "x": np.ascontiguousarray(x[i * BPC:(i + 1) * BPC]),
                "w": w.reshape(C, 1),
                "a": a.reshape(C, 1),
                "wi": wi.reshape(C, 1),
            }
        )
    return in_maps


def kernel(x, weights):
    nc = _get_nc()
    in_maps = make_in_maps(x, weights)
    res = run_bass_kernel_spmd(nc, in_maps, list(range(N_CORES)))
    return np.concatenate([r["y"] for r in res.results], axis=0)
